# revision 57
# baseline (speedup 1.0000x reference)
"""Trainium2 Bass kernel for the faithful-reshape causal attention module.

Math (per the reference's raw row-major reshape [B,L,3D] -> [B,H,L,192]):
block (b, h) consumes x rows [128h, 128h+128) of batch b only:
  qkv   = x_blk @ Wqkv                     # [128, 3072]
  q,k,v = qkv.reshape(2048, 192) split     # pseudo-positions m = 16a + r
  S     = (q @ k^T) / 4, causal over m
  o     = softmax(S) @ v  -> reshape [128, 1024]
  y_blk = o @ Wo
32 independent blocks; 8 cores x 4 blocks, zero collectives.

v2 (causal-skip restructure): q^T/k^T/v^T are materialized in TRUE
pseudo-position order (m = 16a + r), so S^T [kpos, qpos] decomposes into
16 kpos-strips x 4 qpos-chunks of which only the block-lower-triangular
half is live (53% of tiles). Off-diagonal live tiles need no mask at
all; the 16 diagonal 128x128 tiles get a plain tril affine_select.
S/PV run in bf16 (same modeled PE rate as f32r, half the DVE/SBUF
traffic); the qkv and Wo gemms stay f32r with weights DMA'd via dtype
bitcast (zero cast traffic). q^T/k^T come from SP-issued DMA-transposes
(bf16) + one strided re-layout copy each on Pool; v^T via PE transposes
with strided m-order evictions, then per-strip back-transposes into
[kpos, c] form for PV.
"""
import sys

sys.path.insert(0, '/opt/trn_rl_repo')

import numpy as np

B, L, D = 2, 2048, 1024
H = 16              # heads == blocks per batch
RB = 128            # x rows per block
D3 = 3 * D
NR = 16             # r-groups (192-col chunks per row)
NB = 4              # blocks per core
NCORES = 8
P = 128
NCH = 6             # Wqkv 512-col streaming chunks
NC = 4              # qpos chunks of 512 per block
NT = 16             # kpos strips of 128 per block

_cached = {}


def _build_program():
    import concourse.bass as bass
    import concourse.mybir as mybir
    import concourse.tile as tile
    from concourse.tile import add_dep_helper

    f32 = mybir.dt.float32
    f32r = mybir.dt.float32r
    bf16 = mybir.dt.bfloat16
    EXP = mybir.ActivationFunctionType.Exp
    GE = mybir.AluOpType.is_ge

    nc = bass.Bass()
    xs = nc.declare_dram_parameter("xs", [NB, RB, D], f32, isOutput=False)
    wqkv = nc.declare_dram_parameter("wqkv", [D, D3], f32, isOutput=False)
    wo = nc.declare_dram_parameter("wo", [D, D], f32, isOutput=False)
    ys = nc.declare_dram_parameter("ys", [NB, RB, D], f32, isOutput=True)
    DBG = _cached.get("debug", False)
    if DBG:
        d_qkv = nc.declare_dram_parameter("d_qkv", [P, D3], f32, isOutput=True)
        d_qT = nc.declare_dram_parameter("d_qT", [64, L], f32, isOutput=True)
        d_kT = nc.declare_dram_parameter("d_kT", [64, L], f32, isOutput=True)
        d_vT = nc.declare_dram_parameter("d_vT", [64, L], f32, isOutput=True)
        d_va = nc.declare_dram_parameter("d_va", [P, NT * 65], f32, isOutput=True)
        d_pt = nc.declare_dram_parameter("d_pt", [P, 1024], f32, isOutput=True)
        d_osb = nc.declare_dram_parameter("d_osb", [64, 512], f32, isOutput=True)
        d_qkvT = nc.declare_dram_parameter("d_qkvT", [P, 3072], f32, isOutput=True)

    from contextlib import ExitStack
    with tile.TileContext(nc) as tc:
        with ExitStack() as _stk:
            def _pool(**kw):
                return _stk.enter_context(tc.tile_pool(**kw))

            constp = _pool(name="const", bufs=1)
            wqp = _pool(name="wq", bufs=3)
            wop = _pool(name="wop", bufs=1)
            xp = _pool(name="xp", bufs=2)
            xtp = _pool(name="xtp", bufs=1)
            qkvp = _pool(name="qkvp", bufs=1)
            qtmp = _pool(name="qtm", bufs=3)
            ktmp = _pool(name="ktm", bufs=3)
            vtmp = _pool(name="vtm", bufs=3)
            vaugp = _pool(name="vaug", bufs=3)
            wlp = _pool(name="wl", bufs=2)
            ptp = _pool(name="pt", bufs=4)
            nrmp = _pool(name="nrm", bufs=2)
            yop = _pool(name="yo", bufs=2)
            qpsp = _pool(name="qps", bufs=2, space="PSUM")
            sttp = _pool(name="stt", bufs=2, space="PSUM")
            otqp = _pool(name="otq", bufs=2, space="PSUM")
            _pend_nops = []

            def absorb_on(eng, *prods):
                # Walrus caps every instruction at ONE sync wait. Emit
                # queue-local nops that sync-depend on each producer; the
                # post-pass elides waits covered by these earlier nops.
                for p in prods:
                    if p is None:
                        continue
                    n = eng.nop(hint="dep")
                    add_dep_helper(n.ins, p.ins, sync=True)
                    _pend_nops.append(n)

            def pin(h):
                # keep absorber nops scheduled before their instruction
                while _pend_nops:
                    n = _pend_nops.pop()
                    add_dep_helper(h.ins, n.ins, sync=False)
                return h

            dma_hs = []

            def dma(eng, dst, src, *deps):
                _pend_nops.clear()
                absorb_on(eng, *deps)
                h = pin(eng.dma_start(dst, src))
                dma_hs.append(h)
                return h

            # shared PSUM ring [128, 512] f32 for qkv-gemm outs, packed
            # transposes, bc broadcasts and wo-gemm outs
            qps_readers = []
            qps_n = [0]

            def qps_tile():
                n = qps_n[0]
                if n >= 2:
                    absorb_on(nc.tensor, qps_readers[n - 2])
                qps_n[0] += 1
                return qpsp.tile([P, 512], f32, tag="qps", name="qpstile")

            # ---- constants
            identb = constp.tile([P, 128], bf16, tag="identb")
            h_idm = nc.gpsimd.memset(identb[:], 0.0)
            absorb_on(nc.gpsimd, h_idm)
            h_idb = nc.gpsimd.affine_select(
                out=identb[:], in_=identb[:],
                compare_op=mybir.AluOpType.not_equal,
                fill=1.0, base=0, pattern=[[-1, 128]], channel_multiplier=1)
            identr_f = constp.tile([P, 128], f32, tag="identr")
            h_idm2 = nc.gpsimd.memset(identr_f[:], 0.0)
            absorb_on(nc.gpsimd, h_idm2)
            h_idr = nc.gpsimd.affine_select(
                out=identr_f[:], in_=identr_f[:],
                compare_op=mybir.AluOpType.not_equal,
                fill=1.0, base=0, pattern=[[-1, 128]], channel_multiplier=1)
            identr = identr_f[:]
            ones_f = constp.tile([1, 128], f32, tag="onesf")
            h_of = nc.gpsimd.memset(ones_f[:], 1.0)
            onesr = constp.tile([1, 128], f32r, tag="onesr")
            absorb_on(nc.vector, h_of)
            h_ones = nc.vector.tensor_copy(onesr[:], ones_f[:])

            # ---- phase A: x load + f32r transpose -> xT
            xT = xtp.tile([P, NB, 8, P], f32r, tag="xT")
            x_ev = []           # per block: last xT eviction handle (DVE)
            x_tr = []           # per block: last x transpose handle (PE)
            h_xev = None
            for i in range(NB):
                x_sb = xp.tile([P, D], f32, tag="x")
                h_x = dma(nc.gpsimd, x_sb[:], xs[i],
                          x_tr[i - 2] if i >= 2 else None)
                absorb_on(nc.tensor, h_x, h_idr if i == 0 else None)
                for g in range(2):
                    tp = qps_tile()
                    h_tr = None
                    for j in range(4):
                        h_tr = nc.tensor.transpose(
                            tp[:, 128 * j:128 * j + 128],
                            x_sb[:, 512 * g + 128 * j:512 * g + 128 * j + 128],
                            identr)
                    absorb_on(nc.vector, h_tr)
                    h_xev = nc.vector.tensor_copy(
                        xT[:, i, 4 * g:4 * g + 4, :],
                        tp[:].rearrange("p (k a) -> p k a", k=4))
                    qps_readers.append(h_xev)
                x_ev.append(h_xev)
                x_tr.append(h_tr)

            # ---- phase A: stream Wqkv (f32r bitcast, no casts), qkv gemms,
            # and per-block span DMA-transposes (3 per block, interleaved)
            qkv = qkvp.tile([P, NB, D3], bf16, tag="qkv")
            qkvT = qkvp.tile([P, NB, 24, P], bf16, tag="qkvT")
            evict_h = {}        # (nch, i) -> eviction handle
            span_h = {}         # (i, w) -> dma-transpose handle
            wq_hist = []
            def emit_spans(w):
                for i in range(NB):
                    _pend_nops.clear()
                    absorb_on(nc.sync, evict_h[(6 * w + 5, i)])
                    h_sp = pin(nc.sync.dma_start_transpose(
                        qkvT[:, i, 12 * w:12 * w + 12, :],
                        qkv[:, i, 1536 * w:1536 * w + 1536]))
                    dma_hs.append(h_sp)
                    span_h[(i, w)] = h_sp

            for nch in range(12):
                if nch == 7:
                    emit_spans(0)
                wq_t = wqp.tile([P, 8, 256], f32r, tag="wq")
                h_wd = dma(
                    nc.gpsimd, wq_t[:],
                    wqkv.rearrange("(kc p) n -> p kc n", p=P)
                    [:, :, 256 * nch:256 * nch + 256].bitcast(f32r),
                    wq_hist[-3] if len(wq_hist) >= 3 else None,
                    *span_h.values(), *dma_hs[-9:-6])
                h_mm = None
                for i in range(NB):
                    qp = qps_tile()
                    absorb_on(nc.tensor, h_wd if i == 0 else None,
                              x_ev[i] if nch == 0 else None)
                    for k in range(8):
                        h_mm = nc.tensor.matmul(
                            qp[:, 0:256],
                            xT[:, i, k, :],
                            wq_t[:, k, :],
                            start=(k == 0), stop=(k == 7),
                        )
                    absorb_on(nc.vector, h_mm)
                    h_ev = nc.vector.tensor_copy(
                        qkv[:, i, 256 * nch:256 * nch + 256], qp[:, 0:256])
                    qps_readers.append(h_ev)
                    evict_h[(nch, i)] = h_ev
                wq_hist.append(h_mm)
            emit_spans(1)

            # ---- phase A: Wo resident (f32r bitcast)
            wo_sb = wop.tile([P, 8, D], f32r, tag="wo")
            h_wo_dma = []
            for wc in range(2):
                h = dma(
                    nc.gpsimd,
                    wo_sb[:, :, 512 * wc:512 * wc + 512],
                    wo.rearrange("(kc p) n -> p kc n", p=P)
                    [:, :, 512 * wc:512 * wc + 512].bitcast(f32r),
                    h_wo_dma[0] if wc == 1 else None,
                    *[h for h in span_h.values()][-8:])
                h_wo_dma.append(h)

            # ---- per-block setup pieces (emitted interleaved)
            def strip_width(t, c):
                j = t - 4 * c
                return 512 if j < 0 else 512 - 128 * j

            blk = {}            # i -> dict of tiles/handles
            pt_ring = []        # (exp, last_pv, last_writer) per pt use
            pt_n = [0]

            # six m-order re-layout copies per block, from the 128-col slot
            # decomposition: global col g = 128*S + p; q/k/v of r-group r sit
            # at (partition base, slot offset mod 3) per r parity.
            #   name: (dst, src_pbase, slot_off, r0, engine)
            REL = [
                ('qT', 0, 0, 0), ('kT', 64, 0, 0), ('vT', 0, 1, 0),
                ('qT', 64, 1, 1), ('kT', 0, 2, 1), ('vT', 64, 2, 1),
            ]

            def war_pe():
                return pt_ring[-1][1] if pt_ring else None

            def emit_relayout(i, idx):
                b = blk.setdefault(i, {})
                if 'qT' not in b:
                    b['qT'] = qtmp.tile([64, L], bf16, tag="qT", name="qTm")
                    b['kT'] = ktmp.tile([64, L], bf16, tag="kT", name="kTm")
                    b['vT'] = vtmp.tile([64, L], bf16, tag="vT", name="vTm")
                    b['rel'] = {}
                name, pbase, off, r0 = REL[idx]
                eng = nc.gpsimd if name == 'kT' else nc.vector
                absorb_on(eng, span_h[(i, 0)], span_h[(i, 1)],
                          war_pe() if i >= 2 else None)
                h = eng.tensor_copy(
                    b[name][:].rearrange("p (a r) -> p a r", r=NR)
                    [:, :, r0:NR:2],
                    qkvT[pbase:pbase + 64, i, off:24:3, :]
                    .rearrange("p u a -> p a u"))
                b['rel'][(name, r0)] = h
                return h

            def emit_vaug(i, t_lo, t_hi):
                b = blk[i]
                if 'vaug' not in b:
                    b['vaug'] = vaugp.tile([P, NT, 65], bf16, tag="vaug", name="vaug")
                    # ones live only in col 64, which later blocks' evictions
                    # never touch: initialize the two pool slots once
                    if i < 3:
                        b['h_vm'] = nc.gpsimd.memset(b['vaug'][:], 1.0)
                    else:
                        b['h_vm'] = None
                    b['h_bt_ev'] = [None] * NT
                for t in range(t_lo, t_hi):
                    tp = qps_tile()
                    tpb = tp[:].bitcast(bf16)
                    absorb_on(nc.tensor,
                              b['rel'][('vT', 0)] if t == t_lo else None,
                              b['rel'][('vT', 1)] if t == t_lo else None)
                    h_bt = nc.tensor.transpose(
                        tpb[:, 0:64],
                        b['vT'][:, 128 * t:128 * t + 128],
                        identb[0:64, 0:64])
                    absorb_on(nc.vector, h_bt,
                              b['h_vm'] if t == t_lo else None)
                    h_be = nc.vector.tensor_copy(
                        b['vaug'][:, t, 0:64], tpb[:, 0:64])
                    qps_readers.append(h_be)
                    b['h_bt_ev'][t] = h_be
                    b['h_bt'] = h_bt

            # ---- attention per block, c-major with strip pairs
            stt_ring = []       # exp handle per stt tile use
            stt_n = [0]
            otq_readers = []    # per otq alloc: a normalization reader
            otq_n = [0]
            mul_hist = []
            y_hist = []
            pending_tail = []
            tail_last = {}

            def emit_tail(ti, t_wo_lhsT, t_h_mul):
                absorb_on(nc.tensor, t_h_mul,
                          h_wo_dma[1] if ti == 0 else None)
                y_sb = yop.tile([P, D], f32, tag="y")
                h_ye = None
                for n2 in range(2):
                    yp = qps_tile()
                    for k in range(8):
                        tail_last["womm"] = nc.tensor.matmul(
                            yp[:, 0:512],
                            t_wo_lhsT[:, k, :],
                            wo_sb[:, k, 512 * n2:512 * n2 + 512],
                            start=(k == 0), stop=(k == 7),
                        )
                    absorb_on(nc.vector, tail_last["womm"],
                              y_hist[-1][1] if (n2 == 0 and y_hist) else None)
                    h_ye = nc.vector.tensor_copy(
                        y_sb[:, 512 * n2:512 * n2 + 512], yp[:, 0:512])
                    tail_last["ye"] = h_ye
                    qps_readers.append(h_ye)
                    h_yd = dma(nc.gpsimd,
                               ys[ti].rearrange("p d -> p d")[:, 512 * n2:512 * n2 + 512],
                               y_sb[:, 512 * n2:512 * n2 + 512], h_ye,
                               *[h for h in span_h.values()][-2:])
                y_hist.append((h_ye, h_yd))

            # setups for blocks 0 and 1 before the attention loop
            for idx in range(6):
                emit_relayout(0, idx)
                emit_relayout(1, idx)
            emit_vaug(0, 0, NT)
            emit_vaug(1, 0, NT)

            h_exp = h_pv = h_mul = None
            wo_tiles = {}
            for A in (0, 2):
                B = A + 1
                for X in (A, B):
                    wo_tiles[X] = wlp.tile([P, 8, P], f32r, tag="wl",
                                           name="wl")
                while pending_tail:
                    emit_tail(*pending_tail.pop(0))
                for X in (A, B):
                    bX = blk[X]
                    absorb_on(nc.tensor, bX['rel'][('qT', 0)],
                              bX['rel'][('qT', 1)], bX['rel'][('kT', 0)],
                              bX['rel'][('kT', 1)])
                h_mul_blk = {}
                for c in range(NC):
                    # interleave next pair's setup at chunk boundaries
                    if A == 0:
                        if c == 0:
                            for idx in range(6):
                                emit_relayout(2, idx)
                        elif c == 1:
                            emit_vaug(2, 0, NT)
                        elif c == 2:
                            for idx in range(6):
                                emit_relayout(3, idx)
                        elif c == 3:
                            emit_vaug(3, 0, NT)
                    cm0 = 512 * c
                    tmax = 4 * c + 3
                    for X in (A, B):
                        bX = blk[X]
                        qT, kT, v_aug = bX['qT'], bX['kT'], bX['vaug']
                        h_bt_ev = bX['h_bt_ev']
                        if otq_n[0] >= 2:
                            absorb_on(nc.tensor, otq_readers[otq_n[0] - 2])
                        otq_n[0] += 1
                        otq = otqp.tile([65, 512], f32, tag="otq", name="otq")
                        for t0 in range(0, tmax + 1, 2):
                            pair = [t for t in (t0, t0 + 1) if t <= tmax]
                            widths = [strip_width(t, c) for t in pair]
                            if stt_n[0] >= 2:
                                absorb_on(nc.tensor, stt_ring[stt_n[0] - 2])
                            stt_n[0] += 1
                            stt = sttp.tile([P, 1024], f32, tag="stt")
                            off = 0
                            offs = []
                            h_s = None
                            for t, w in zip(pair, widths):
                                absorb_on(nc.tensor,
                                          h_bt_ev[t] if c == t // 4 else None)
                                h_s = nc.tensor.matmul(
                                    stt[:, off:off + w],
                                    kT[:, 128 * t:128 * t + 128],
                                    qT[:, cm0 + 512 - w:cm0 + 512],
                                    start=True, stop=True,
                                )
                                offs.append(off)
                                off += w
                            if pt_n[0] >= 4:
                                absorb_on(nc.scalar, pt_ring[pt_n[0] - 4][1],
                                          pt_ring[pt_n[0] - 4][2])
                            pt = ptp.tile([P, 1024], bf16, tag="pt")
                            pt_n[0] += 1
                            absorb_on(nc.scalar, h_s)
                            h_exp = nc.scalar.activation(
                                pt[:, 0:off], stt[:, 0:off], EXP, scale=0.25)
                            stt_ring.append(h_exp)
                            h_mask = {}
                            h_lastw = h_exp
                            for t, w, o in zip(pair, widths, offs):
                                if t >= 4 * c:
                                    absorb_on(nc.gpsimd, h_exp)
                                    h_mask[t] = nc.gpsimd.affine_select(
                                        out=pt[:, o:o + 128],
                                        in_=pt[:, o:o + 128],
                                        compare_op=GE, fill=0.0, base=0,
                                        pattern=[[1, 128]],
                                        channel_multiplier=-1)
                                    h_lastw = h_mask[t]
                            for t, w, o in zip(pair, widths, offs):
                                absorb_on(nc.tensor, h_mask.get(t, h_exp))
                                h_pv = nc.tensor.matmul(
                                    otq[:, 512 - w:512],
                                    v_aug[:, t, :],
                                    pt[:, o:o + w],
                                    start=(t == 0), stop=(t == tmax),
                                )
                            pt_ring.append((h_exp, h_pv, h_lastw))
                        # normalization + wo_lhsT build for this chunk
                        rcp = nrmp.tile([1, 512], f32r, tag="rcp")
                        absorb_on(nc.vector, h_pv)
                        with nc.allow_low_precision(
                                reason="f32r rounding of 1/d"):
                            h_rcp = nc.vector.reciprocal(rcp[:], otq[64:65, :])
                        o_sb = nrmp.tile([64, 512], bf16, tag="osb")
                        h_osb = nc.vector.tensor_copy(o_sb[:], otq[0:64, :])
                        otq_readers.append(h_osb)
                        bc = qps_tile()
                        absorb_on(nc.tensor, h_rcp)
                        h_bc = nc.tensor.matmul(
                            bc[:, 0:512], onesr[:], rcp[:, 0:512],
                            start=True, stop=True,
                        )
                        absorb_on(nc.vector, h_bc)
                        for par in range(2):
                            s_ap = o_sb[:, par:512:2].rearrange(
                                "p (a ch) -> p ch a", ch=8)
                            bsr = bc[0:64, par:512:2].rearrange(
                                "p (a ch) -> p ch a", ch=8)
                            h_mul = nc.vector.tensor_mul(
                                wo_tiles[X][64 * par:64 * par + 64, :,
                                            32 * c:32 * c + 32],
                                s_ap, bsr)
                            mul_hist.append(h_mul)
                            qps_readers.append(h_mul)
                            otq_readers.append(h_mul)
                        h_mul_blk[X] = h_mul
                for X in (A, B):
                    pending_tail.append((X, wo_tiles[X], h_mul_blk[X]))

            while pending_tail:
                emit_tail(*pending_tail.pop(0))

            # absorb the kernel-tail drain's dependencies onto SP nops
            absorb_on(nc.sync, *dma_hs)
            absorb_on(nc.sync, tail_last["ye"], tail_last["womm"],
                      h_mul, h_pv, h_exp, h_ones)

    return nc


def _elide_covered_waits(nc):
    """Walrus rejects >1 sync wait per instruction. Each queue's sequencer
    processes waits in dispatch order, so a wait already issued earlier in
    the same queue gates every later instruction in that queue. Drop waits
    that an earlier same-queue instruction (incl. absorber nops) covers."""
    observed = {}   # engine -> {sem_id: max waited value}
    leftover = []
    for inst in nc.all_instructions():
        si = inst.sync_info
        if si is None:
            continue
        if type(inst).__name__ in ("InstEventSemaphore", "InstTrigger"):
            continue  # barrier-protocol handshakes, not data waits
        eng = str(inst.engine)
        short = eng.split(".")[-1]
        obs = observed.setdefault(eng, {})
        ow = list(si.on_wait or [])
        keep = []
        for w in ow:
            if obs.get(w.id, -1) >= w.wait_value:
                continue
            if w.ant_name.startswith(short + "_"):
                # wait on this engine's own completion counter: satisfied
                # by in-order execution of the same queue
                obs[w.id] = max(obs.get(w.id, -1), w.wait_value)
                continue
            keep.append(w)
            obs[w.id] = max(obs.get(w.id, -1), w.wait_value)
        if len(keep) != len(ow):
            si.on_wait = keep
            inst.sync_info = si
        if len(keep) > 1:
            leftover.append((inst.name, type(inst).__name__, eng,
                             [(w.ant_name, w.wait_value) for w in keep]))
    if leftover:
        import logging
        logging.warning("multi-wait instructions remain: %s", leftover[:12])


def _get_program():
    if "nc" not in _cached:
        nc = _build_program()
        _elide_covered_waits(nc)
        _cached["nc"] = nc
    return _cached["nc"]


def kernel(x=None, mask=None, Wqkv=None, Wo=None, **_ignored):
    """Full inputs -> full output. mask is ignored (guaranteed causal tril)."""
    from concourse.bass_utils import run_bass_kernel_spmd

    x = np.ascontiguousarray(np.asarray(x, dtype=np.float32))
    Wqkv = np.ascontiguousarray(np.asarray(Wqkv, dtype=np.float32))
    Wo = np.ascontiguousarray(np.asarray(Wo, dtype=np.float32))

    nc = _get_program()
    in_maps = []
    for c in range(NCORES):
        shards = []
        for g in range(NB * c, NB * c + NB):
            b, h = divmod(g, H)
            shards.append(x[b, RB * h:RB * h + RB, :])
        in_maps.append({
            "xs": np.ascontiguousarray(np.stack(shards, axis=0)),
            "wqkv": Wqkv,
            "wo": Wo,
        })

    res = run_bass_kernel_spmd(nc, in_maps, core_ids=list(range(NCORES)))
    y = np.empty((B, L, D), dtype=np.float32)
    for c in range(NCORES):
        ysc = res.results[c]["ys"]
        for idx, g in enumerate(range(NB * c, NB * c + NB)):
            b, h = divmod(g, H)
            y[b, RB * h:RB * h + RB, :] = ysc[idx]
    return y
